# revision 16
# baseline (speedup 1.0000x reference)
"""Trainium2 Bass kernel for a 3-layer bidirectional projected-LSTM embedder.

Model (from the reference):
  T=160, B=640, F=40, HID=768, PROJ=256, 3 stacked LSTM-with-projection
  layers per direction (fw, bw).  Per step:
      z = [x_t, h_{t-1}] @ Wk + b            # [B, 4*HID], gate order i,j,f,o
      c = sig(f+1)*c + sig(i)*tanh(j)
      h = (sig(o)*tanh(c)) @ Wp              # [B, PROJ]
  Output = l2norm((concat(fw,bw)[t=0] + concat(fw,bw)[t=T-1]) / 2)  # [B, 512]

Strategy: pure data-parallel over batch (80 per core, 8 cores, no
collectives).  Per core, the three layers run as sequential phases; within a
phase the fw and bw recurrences are interleaved so PE/ACT/DVE overlap.  All
matmuls use float32r (full-rate PE, ~7e-6 elementwise rounding).  Batch-major
z = lhsT.T @ Wk with the activations as the stationary operand and the
(SBUF-resident) weights streaming.  Per-step PE transposes produce the
gate-major h^T needed as next-step stationary operand.  Layer-to-layer h
sequences ping-pong through DRAM.  The final (t0+tT)/2 + l2-normalize is done
on the host in numpy.
"""

import numpy as np

T, B, F = 160, 640, 40
HID, PROJ = 768, 256
NG = 4 * HID          # 3072
NCORES = 8
BC = B // NCORES      # 80
NKH = PROJ // 128     # 2 k-tiles for the recurrent part

_BUILD_CACHE = {}

# Wk column permutation: gate order i,j,f,o -> [i | o | f | j]
_WK_PERM = np.concatenate([np.arange(0, 768), np.arange(2304, 3072), np.arange(1536, 2304), np.arange(768, 1536)])


def _build(use_bias, t_steps, cw=512):
    from contextlib import ExitStack

    import concourse.bass as bass  # noqa: F401
    import concourse.tile as tile
    from concourse import bacc, mybir
    from concourse.masks import make_identity

    f32 = mybir.dt.float32
    f32r = mybir.dt.float32r
    AF = mybir.ActivationFunctionType

    DIRS = ("fw", "bw")
    CW = cw

    nc = bacc.Bacc(None, target_bir_lowering=False)

    xT = nc.declare_dram_parameter("xT", [F, t_steps * BC], f32r, isOutput=False)
    wk_in = {}
    wp_in = {}
    bb_in = {}
    for d in DIRS:
        for l in range(3):
            ind = F if l == 0 else PROJ
            wk_in[d, l] = nc.declare_dram_parameter(
                f"Wk_{d}{l}", [ind + PROJ, NG], f32r, isOutput=False)
            wp_in[d, l] = nc.declare_dram_parameter(
                f"Wp_{d}{l}", [HID, PROJ], f32r, isOutput=False)
            if use_bias:
                bb_in[d, l] = nc.declare_dram_parameter(
                    f"bb_{d}{l}", [BC, NG], f32, isOutput=False)
    # hT of the top layer at t=0 and t=T-1:  [dir, end, 128, kt, BC]
    out_ends = nc.declare_dram_parameter(
        "out_ends", [2, 2, 128, NKH, BC], f32r, isOutput=True)

    with tile.TileContext(nc) as tc:
        with ExitStack() as top:
            glob = top.enter_context(tc.tile_pool(name="glob", bufs=1))
            dram = top.enter_context(tc.tile_pool(name="dram", bufs=1, space="DRAM"))

            ident = glob.tile([BC, BC], f32)
            make_identity(nc, ident)

            # layer-to-layer h^T sequences (ping-pong per direction)
            hseq = {}
            for d in DIRS:
                for i in (0, 1):
                    hseq[d, i] = dram.tile([128, NKH, t_steps, BC], f32r,
                                           name=f"hseq_{d}{i}", tag=f"hseq_{d}{i}")

            for l in range(3):
                in_dim = F if l == 0 else PROJ
                with ExitStack() as ph:
                    wpool = ph.enter_context(tc.tile_pool(name=f"w{l}", bufs=1))
                    spool = ph.enter_context(tc.tile_pool(name=f"s{l}", bufs=1))
                    gpool = ph.enter_context(tc.tile_pool(name=f"g{l}", bufs=1))
                    xpool = ph.enter_context(tc.tile_pool(name=f"x{l}", bufs=6))
                    zpool = ph.enter_context(
                        tc.tile_pool(name=f"z{l}", bufs=1, space="PSUM"))
                    apool = ph.enter_context(
                        tc.tile_pool(name=f"a{l}", bufs=1, space="PSUM"))

                    # ---- load weights into SBUF ----
                    # k-tile row spans of Wk: x-part rows then h-part rows
                    if l == 0:
                        kspans = [(0, F), (F, 128), (F + 128, 128)]
                    else:
                        kspans = [(0, 128), (128, 128), (256, 128), (384, 128)]
                    wk_t = {d: [] for d in DIRS}
                    wp_t = {d: [] for d in DIRS}
                    bb_t = {}
                    for d in DIRS:
                        for ki, (r0, rc) in enumerate(kspans):
                            wt = wpool.tile([rc, NG], f32r,
                                            name=f"wk_{d}{l}_{ki}",
                                            tag=f"wk_{d}_{ki}")
                            for c in range(6):
                                nc.sync.dma_start(
                                    out=wt[:, c * 512:(c + 1) * 512],
                                    in_=wk_in[d, l][r0:r0 + rc,
                                                    c * 512:(c + 1) * 512])
                            wk_t[d].append(wt)
                        for ki in range(6):
                            pt = wpool.tile([128, PROJ], f32r,
                                            name=f"wp_{d}{l}_{ki}",
                                            tag=f"wp_{d}_{ki}")
                            nc.sync.dma_start(
                                out=pt, in_=wp_in[d, l][ki * 128:(ki + 1) * 128, :])
                            wp_t[d].append(pt)
                        if use_bias:
                            bt = wpool.tile([BC, NG], f32, name=f"bb_{d}{l}",
                                            tag=f"bb_{d}")
                            for c in range(6):
                                nc.sync.dma_start(
                                    out=bt[:, c * 512:(c + 1) * 512],
                                    in_=bb_in[d, l][:, c * 512:(c + 1) * 512])
                            bb_t[d] = bt

                    # ---- state ----
                    st = {}
                    for d in DIRS:
                        c_sb = spool.tile([BC, HID], f32, name=f"c_{d}{l}",
                                          tag=f"c_{d}")
                        st[d] = [c_sb, None]   # hT produced by step 0

                    for step in range(t_steps):
                        for d in DIRS:
                            t = step if d == "fw" else t_steps - 1 - step
                            c_sb, hT = st[d]

                            if l == 0:
                                xin0 = xpool.tile([F, BC], f32r,
                                                  name=f"xin0_{d}", tag=f"xin_{d}")
                                nc.sync.dma_start(
                                    out=xin0, in_=xT[:, t * BC:(t + 1) * BC])
                                xparts = [xin0]
                            else:
                                xin = xpool.tile([128, NKH * BC], f32r,
                                                 name=f"xin_{d}{l}",
                                                 tag=f"xin_{d}")
                                nc.sync.dma_start(
                                    out=xin.rearrange("p (k b) -> p k b", k=NKH),
                                    in_=hseq[d, (l - 1) % 2][:, :, t, :])
                                xparts = [xin[:, ki * BC:(ki + 1) * BC]
                                          for ki in range(NKH)]
                            if step == 0:
                                lhsts = xparts   # h_{-1} = 0
                            else:
                                lhsts = xparts + [hT[:, ki * BC:(ki + 1) * BC]
                                                  for ki in range(NKH)]

                            # z = [x, h] @ Wk  -> chunks of [BC, CW] in PSUM
                            nch = NG // CW
                            zc = []
                            for c in range(nch):
                                zt = zpool.tile([BC, CW], f32,
                                                name=f"z{c}_{d}{l}", tag=f"z{c}")
                                for ns in range(CW // 512):
                                    cols = slice(c * CW + ns * 512,
                                                 c * CW + (ns + 1) * 512)
                                    for ki, lt in enumerate(lhsts):
                                        nc.tensor.matmul(
                                            zt[:, ns * 512:(ns + 1) * 512],
                                            lt, wk_t[d][ki][:, cols],
                                            start=(ki == 0),
                                            stop=(ki == len(lhsts) - 1))
                                zc.append(zt)

                            # gates (gate g spans z cols [g*HID, (g+1)*HID))
                            gt = {}
                            for g, fn, bias in ((0, AF.Sigmoid, 0.0),
                                                (1, AF.Tanh, 0.0),
                                                (2, AF.Sigmoid, 1.0),
                                                (3, AF.Sigmoid, 0.0)):
                                gt[g] = gpool.tile([BC, HID], f32,
                                                   name=f"g{g}_{d}{l}",
                                                   tag=f"g{g}_{d}")
                                glo, ghi = g * HID, (g + 1) * HID
                                for c in range(glo // CW, (ghi - 1) // CW + 1):
                                    lo, hi = max(glo, c * CW), min(ghi, (c + 1) * CW)
                                    gsrc = zc[c][:, lo - c * CW:hi - c * CW]
                                    if use_bias:
                                        tb = gpool.tile([BC, HID], f32,
                                                        name=f"tb_{d}{l}",
                                                        tag=f"tb_{d}")
                                        nc.vector.tensor_add(
                                            tb[:, 0:hi - lo], gsrc,
                                            bb_t[d][:, lo:hi])
                                        gsrc = tb[:, 0:hi - lo]
                                    nc.scalar.activation(
                                        gt[g][:, lo - glo:hi - glo], gsrc,
                                        fn, bias=bias)

                            # c = sig(f+1)*c + sig(i)*tanh(j)
                            if step == 0:
                                nc.vector.tensor_mul(c_sb, gt[0], gt[1])
                            else:
                                tmp = gpool.tile([BC, HID], f32,
                                                 name=f"tmp_{d}{l}", tag=f"tmp_{d}")
                                nc.vector.tensor_mul(tmp, gt[0], gt[1])
                                nc.vector.tensor_mul(c_sb, gt[2], c_sb)
                                nc.vector.tensor_add(c_sb, c_sb, tmp)
                            tanhc = gpool.tile([BC, HID], f32,
                                               name=f"tanhc_{d}{l}",
                                               tag=f"tanhc_{d}")
                            nc.scalar.activation(tanhc, c_sb, AF.Tanh)
                            s_sb = gpool.tile([BC, HID], f32,
                                              name=f"s_{d}{l}", tag=f"s_{d}")
                            nc.vector.tensor_mul(s_sb, gt[3], tanhc)

                            # s^T via PE transposes -> [768(6x128), BC]
                            sT_ps = apool.tile([128, 6 * BC], f32,
                                               name=f"sTp_{d}{l}", tag="sT")
                            for j in range(6):
                                nc.tensor.transpose(
                                    sT_ps[:, j * BC:(j + 1) * BC],
                                    s_sb[:, j * 128:(j + 1) * 128], ident)
                            sT_sb = gpool.tile([128, 6 * BC], f32r,
                                               name=f"sT_{d}{l}", tag=f"sT_{d}")
                            nc.vector.tensor_copy(sT_sb, sT_ps)

                            # h = s @ Wp  [BC, PROJ], then h^T [256(2x128), BC]
                            aux = apool.tile([128, 512], f32,
                                             name=f"aux_{d}{l}", tag="aux")
                            h_ps = aux[0:BC, 0:PROJ]
                            for ki in range(6):
                                nc.tensor.matmul(
                                    h_ps, sT_sb[:, ki * BC:(ki + 1) * BC],
                                    wp_t[d][ki], start=(ki == 0), stop=(ki == 5))
                            h_sb = gpool.tile([BC, PROJ], f32,
                                              name=f"h_{d}{l}", tag=f"h_{d}")
                            nc.vector.tensor_copy(h_sb, h_ps)
                            for j in range(NKH):
                                nc.tensor.transpose(
                                    aux[:, PROJ + j * BC:PROJ + (j + 1) * BC],
                                    h_sb[:, j * 128:(j + 1) * 128], ident)
                            hT_new = spool.tile([128, NKH * BC], f32r,
                                                name=f"hTn_{d}{l}", tag=f"hT_{d}")
                            nc.vector.tensor_copy(hT_new, aux[:, PROJ:PROJ + NKH * BC])
                            st[d][1] = hT_new

                            if l < 2:
                                nc.sync.dma_start(
                                    out=hseq[d, l % 2][:, :, t, :],
                                    in_=hT_new.rearrange("p (k b) -> p k b", k=NKH))
                            else:
                                di = 0 if d == "fw" else 1
                                if t == 0:
                                    nc.sync.dma_start(
                                        out=out_ends[di, 0],
                                        in_=hT_new.rearrange("p (k b) -> p k b",
                                                             k=NKH))
                                if t == t_steps - 1:
                                    nc.sync.dma_start(
                                        out=out_ends[di, 1],
                                        in_=hT_new.rearrange("p (k b) -> p k b",
                                                             k=NKH))

    nc.finalize()
    return nc


def _get_nc(use_bias, t_steps, cw=512):
    key = (use_bias, t_steps, cw)
    if key not in _BUILD_CACHE:
        _BUILD_CACHE[key] = _build(use_bias, t_steps, cw)
    return _BUILD_CACHE[key]


def kernel(**inputs):
    from concourse.bass_utils import run_bass_kernel_spmd

    inp = {k: np.asarray(v, dtype=np.float32) for k, v in inputs.items()}
    batch = inp["batch"]
    assert batch.shape == (T, B, F), batch.shape

    use_bias = any(np.any(inp[f"b_{d}{l}"]) for d in ("fw", "bw") for l in range(3))
    nc = _get_nc(use_bias, T)

    shared = {}
    for d in ("fw", "bw"):
        for l in range(3):
            shared[f"Wk_{d}{l}"] = np.ascontiguousarray(inp[f"Wk_{d}{l}"])
            shared[f"Wp_{d}{l}"] = np.ascontiguousarray(inp[f"Wp_{d}{l}"])
            if use_bias:
                shared[f"bb_{d}{l}"] = np.ascontiguousarray(
                    np.broadcast_to(inp[f"b_{d}{l}"], (BC, NG)))

    in_maps = []
    for i in range(NCORES):
        xb = batch[:, i * BC:(i + 1) * BC, :]           # [T, BC, F]
        xT_i = np.ascontiguousarray(
            xb.transpose(2, 0, 1).reshape(F, T * BC))    # [F, T*BC]
        in_maps.append({"xT": xT_i, **shared})

    res = run_bass_kernel_spmd(nc, in_maps, core_ids=list(range(NCORES)))

    # assemble: out_ends [2(dir), 2(end), 128, NKH, BC] -> h [BC, 256]
    h = np.zeros((2, 2, B, PROJ), dtype=np.float32)    # [dir, end, B, PROJ]
    for i in range(NCORES):
        oe = res.results[i]["out_ends"]
        # h[b, kt*128 + p] = oe[.., p, kt, b]
        h[:, :, i * BC:(i + 1) * BC, :] = oe.transpose(0, 1, 4, 3, 2).reshape(
            2, 2, BC, PROJ)

    out0 = np.concatenate([h[0, 0], h[1, 0]], axis=1)   # t = 0
    outT = np.concatenate([h[0, 1], h[1, 1]], axis=1)   # t = T-1
    emb = (out0 + outT) / np.float32(2.0)
    ss = np.maximum(np.sum(emb * emb, axis=-1, keepdims=True), np.float32(1e-12))
    emb = emb / np.sqrt(ss)
    return emb.astype(np.float32)
